# revision 1
# baseline (speedup 1.0000x reference)
"""Trainium2 Bass kernel for nn_Averager (pooling, 3-level box-average).

Math (verified vs reference): per sample, with input x[n, i, c] where
n = (n5 n4 n3 n2 n1 n0) base-4 digits, c = (c2 c1 c0) base-4 digits:
  out[:, :, 0, :] = x[:, :, 0, :]
  out1[n, c] = E[n4, n2, c2, c0, n0, c1],
      E[r5, r4, r3, r0; g2, g1] = mean over (n2, n1, c0) of x1
  out2[n, c] = G[c2, c1, c0],
      G[p, q, r] = mean over (n4, n3, n1, n0, c1, c0) of x2 with
      (n5, c2in, n2) = (p, q, r)

Sharding: data-parallel over batch, 4 samples per core on 8 cores,
processed as 2 groups of 2 samples.

Layout (pair-contiguous): SBUF partition p = b*64 + n//64 =
(b, n5, n4, n3); free j = n % 64 = 16*n2 + 4*n1 + n0, row (i, c).
A 6MB group is contiguous in DRAM and per-partition contiguous in SBUF,
so each group is ONE 2-D in-DMA and ONE 2-D out-DMA.  All reductions are
lane-local on DVE (reduced digits n2, n1, n0, c1, c0 all live in the
free dim); the PE selector matmuls only route/broadcast E (16 matmuls)
and reduce+broadcast G (4 matmuls) across partitions.  Outputs are
assembled IN-PLACE into the input tile (level regions are dead once the
partial reductions are done), halving SBUF.

Output per group is three region DMAs: L0 straight from the in-tile
(dep: in-DMA), L1 from the DVE-evacuated region (dep: DVE), and L2 via a
step-0 broadcast source AP that replicates the single 64-float G row 64x
during the DMA itself (no on-chip broadcast work).

Hardware constraints honored: every DMA and matmul carries at most ONE
sync wait (pseudo-DMA / LoadWeights structs are single-wait): <= 8 DMAs
per DGE class (SWDGE/HWDGE sem lanes; the L2 out-DMAs ride HWDGE via
nc.sync), <= 3 AP dims per DMA side, zero SBUF/PSUM slot reuse, DVE is
the only engine writing SBUF tiles, and constants are re-copied through
DVE so matmul deps collapse to one sem.
"""

import numpy as np

N_CORES = 8
B_FULL = 32
B_CORE = B_FULL // N_CORES  # 4
N = 4096
LVL = 3
C = 64


def _make_selectors():
    """Routing selectors, pair layout: k = 64*b + 16*k5 + 4*k4 + k3.

    S1 block (n2o, c2o), 16 blocks:
        S1[k, m] = 1/64   iff b(k)==b(m), k5==m4, k4==n2o, k3==c2o
    S2 block (c2o), 4 blocks:
        S2[k, m] = 1/4096 iff b(k)==b(m), k5==c2o
    """
    k = np.arange(128)
    b, k5, k4, k3 = k >> 6, (k >> 4) & 3, (k >> 2) & 3, k & 3
    m = np.arange(128)
    bm, m4 = m >> 6, (m >> 2) & 3
    S1 = np.zeros((128, 16, 128), np.float32)
    S2 = np.zeros((128, 4, 128), np.float32)
    for n2o in range(4):
        for c2o in range(4):
            S1[:, n2o * 4 + c2o, :] = (
                (b[:, None] == bm[None, :])
                & (k5[:, None] == m4[None, :])
                & (k4[:, None] == n2o)
                & (k3[:, None] == c2o)
            ).astype(np.float32) / 64.0
    for c2o in range(4):
        S2[:, c2o, :] = (
            (b[:, None] == bm[None, :]) & (k5[:, None] == c2o)
        ).astype(np.float32) / 4096.0
    return (
        np.ascontiguousarray(S1.reshape(128, 2048)),
        np.ascontiguousarray(S2.reshape(128, 512)),
    )


def _build_nc():
    import concourse.bass as bass
    import concourse.tile as tile
    from concourse import mybir

    dt = mybir.dt.float32
    X = mybir.AxisListType.X
    ADD = mybir.AluOpType.add

    from concourse import bacc
    nc = bacc.Bacc()
    x = nc.declare_dram_parameter("x", [B_CORE, N, LVL, C], dt, isOutput=False)
    s12 = nc.declare_dram_parameter("s12", [128, 2560], dt, isOutput=False)
    out = nc.declare_dram_parameter("out", [B_CORE, N, LVL, C], dt, isOutput=True)

    with tile.TileContext(nc) as tc:
        with (
            tc.tile_pool(name="consts", bufs=1) as cpool,
            tc.tile_pool(name="xin", bufs=2) as xpool,
            tc.tile_pool(name="tmp", bufs=1) as tpool,
            tc.tile_pool(name="psum", bufs=2, space="PSUM") as ppool,
        ):
            s12raw = cpool.tile([128, 2560], dt, tag="s12raw")
            nc.gpsimd.dma_start(s12raw[:], s12[:])
            s12sb = cpool.tile([128, 2560], dt, tag="s12")
            nc.vector.tensor_copy(s12sb[:], s12raw[:])
            s1sb = s12sb[:, 0:2048]
            s2sb = s12sb[:, 2048:2560]

            for g in range(B_CORE // 2):
                bs = slice(2 * g, 2 * g + 2)
                xt = xpool.tile([128, 12288], dt, tag="xt")
                # split the 6MB load so compute (which reads rows j<32 first)
                # starts after the first half lands
                xsrc = x[bs].rearrange("b (ph j) i c -> (b ph) (j i c)", ph=64)
                nc.gpsimd.dma_start(xt[:, 0:6144], xsrc[:, 0:6144])
                nc.gpsimd.dma_start(xt[:, 6144:12288], xsrc[:, 6144:12288])
                xtv = xt[:].rearrange(
                    "p (j i c) -> p j i c", j=64, i=3, c=64
                )

                # ---- L1 stage A: lane-local sum over (n2, n1, c0) ----
                v = xt[:].rearrange(
                    "p (n2 n1 n0 i c) -> p n2 n1 n0 i c",
                    n2=4, n1=4, n0=4, i=3, c=64,
                )
                u0 = tpool.tile([128, 1024], dt, tag="u0")
                nc.vector.tensor_add(
                    u0[:].rearrange("p (n1 n0 c) -> p n1 n0 c", n1=4, n0=4, c=64),
                    v[:, 0, :, :, 1, :], v[:, 1, :, :, 1, :],
                )
                u1 = tpool.tile([128, 1024], dt, tag="u1")
                nc.vector.tensor_add(
                    u1[:].rearrange("p (n1 n0 c) -> p n1 n0 c", n1=4, n0=4, c=64),
                    v[:, 2, :, :, 1, :], v[:, 3, :, :, 1, :],
                )
                w = tpool.tile([128, 1024], dt, tag="w")
                nc.vector.tensor_add(w[:], u0[:], u1[:])
                h1 = tpool.tile([128, 512], dt, tag="h1")
                nc.vector.tensor_add(h1[:], w[:, 0:512], w[:, 512:1024])
                h2 = tpool.tile([128, 256], dt, tag="h2")
                nc.vector.tensor_add(h2[:], h1[:, 0:256], h1[:, 256:512])
                # reduce c0, write A with free = 16*c2 + 4*c1 + n0
                A = tpool.tile([128, 64], dt, tag="A")
                nc.vector.tensor_reduce(
                    A[:].rearrange("p (c2 c1 n0) -> p n0 c2 c1", c2=4, c1=4, n0=4),
                    h2[:].rearrange(
                        "p (n0 c2 c1 c0) -> p n0 c2 c1 c0", n0=4, c2=4, c1=4, c0=4
                    ),
                    axis=X, op=ADD,
                )

                # ---- L1: 16 routing matmuls -> c1p psum (2 banks) ----
                # c1p free = 64*(4*n2o + c2o) + (16*n0o + 4*c1o + c0o)
                c1p = ppool.tile([128, 1024], dt, tag="c1p")
                for n2o in range(4):
                    for c2o in range(4):
                        blk = n2o * 4 + c2o
                        nc.tensor.matmul(
                            c1p[:, blk * 64:(blk + 1) * 64],
                            s1sb[:, blk * 128:(blk + 1) * 128],
                            A[:, 0:64],
                            start=True, stop=True,
                        )
                # ---- L1 evac: 16 copies (n2o, n1o), replicate over n1o ----
                c1e = c1p[:].rearrange(
                    "p (n2o c2o n0 cc) -> p n2o c2o n0 cc",
                    n2o=4, c2o=4, n0=4, cc=16,
                )
                xts = xt[:].rearrange(
                    "p (n2 n1 n0 i c2 cc) -> p n2 n1 c2 n0 i cc",
                    n2=4, n1=4, n0=4, i=3, c2=4, cc=16,
                )
                for n2o in range(4):
                    for n1o in range(4):
                        nc.vector.tensor_copy(
                            xts[:, n2o, n1o, :, :, 1, :],
                            c1e[:, n2o, :, :, :],
                        )

                # ---- L2 stage A: lane-local sums ----
                xw = xt[:].rearrange(
                    "p (j i c2 cc) -> p j i c2 cc", j=64, i=3, c2=4, cc=16
                )
                t4 = tpool.tile([128, 256], dt, tag="t4")
                nc.vector.tensor_reduce(
                    t4[:].rearrange("p (j c2) -> p j c2", j=64, c2=4),
                    xw[:, :, 2, :, :],
                    axis=X, op=ADD,
                )
                A2 = tpool.tile([128, 16], dt, tag="A2")
                nc.vector.tensor_reduce(
                    A2[:].rearrange("p (c2 n2) -> p n2 c2", c2=4, n2=4),
                    t4[:].rearrange(
                        "p (n2 nn c2) -> p n2 c2 nn", n2=4, nn=16, c2=4
                    ),
                    axis=X, op=ADD,
                )

                # ---- L2: 4 reduce+broadcast matmuls -> gp psum ----
                # gp free = 16*c2o + (4*c1o + c0o); rhs j = (c2in, n2)
                gp = ppool.tile([128, 64], dt, tag="gp")
                for c2o in range(4):
                    nc.tensor.matmul(
                        gp[:, c2o * 16:(c2o + 1) * 16],
                        s2sb[:, c2o * 128:(c2o + 1) * 128],
                        A2[:, 0:16],
                        start=True, stop=True,
                    )
                # ---- L2 evac: single row; the out2 DMA broadcasts it ----
                nc.vector.tensor_copy(xtv[:, 0, 2, :], gp[:, 0:64])

                # ---- out: three region DMAs per group ----
                # L0: dep = in-DMA lane; L1: dep = DVE; L2: dep = DVE, src is
                # a step-0 broadcast AP of row 0 (the DMA replicates 64x).
                # out2 goes on HWDGE (nc.sync) lanes to stay within the
                # 8-lane-per-DGE-class budget.
                outv = out[bs].rearrange("b (ph j) i c -> (b ph) j i c", ph=64)
                # HWDGE rings are FIFO per issuing engine: spread the two
                # groups' HWDGE DMAs across both rings (SP and ACT) so they
                # drain concurrently.
                hw = nc.sync if g == 0 else nc.scalar
                hw.dma_start(outv[:, :, 0, :], xtv[:, :, 0, :])
                # L1 out split by row half: the first half flushes while the
                # second half's evacuations finish
                nc.gpsimd.dma_start(outv[:, 0:32, 1, :], xtv[:, 0:32, 1, :])
                hw.dma_start(outv[:, 32:64, 1, :], xtv[:, 32:64, 1, :])
                hw.dma_start(
                    outv[:, :, 2, :],
                    xtv[:, 0:1, 2, :].broadcast_to((128, 64, 64)),
                )
    nc.compile()
    return nc


_NC_CACHE = {}


def _get_nc():
    if "nc" not in _NC_CACHE:
        _NC_CACHE["nc"] = _build_nc()
    return _NC_CACHE["nc"]


def kernel(**inputs: np.ndarray) -> np.ndarray:
    from concourse.bass_utils import run_bass_kernel_spmd

    x = np.ascontiguousarray(inputs["x"], dtype=np.float32)
    assert x.shape == (B_FULL, N, LVL, C), x.shape
    S1, S2 = _make_selectors()
    S12 = np.ascontiguousarray(np.concatenate([S1, S2], axis=1))
    nc = _get_nc()
    in_maps = [
        {"x": np.ascontiguousarray(x[k * B_CORE:(k + 1) * B_CORE]),
         "s12": S12}
        for k in range(N_CORES)
    ]
    res = run_bass_kernel_spmd(nc, in_maps, list(range(N_CORES)))
    outs = [res.results[k]["out"] for k in range(N_CORES)]
    return np.ascontiguousarray(np.concatenate(outs, axis=0))



# revision 3
# speedup vs baseline: 1.3039x; 1.3039x over previous
"""Trainium2 Bass kernel for nn_Averager (pooling, 3-level box-average).

Math (verified vs reference): per sample, with input x[n, i, c] where
n = (n5 n4 n3 n2 n1 n0) base-4 digits, c = (c2 c1 c0) base-4 digits:
  out[:, :, 0, :] = x[:, :, 0, :]
  out1[n, c] = E[n4, n2, c2, c0, n0, c1],
      E[r5, r4, r3, r0; g2, g1] = mean over (n2, n1, c0) of x1
  out2[n, c] = G[c2, c1, c0],
      G[p, q, r] = mean over (n4, n3, n1, n0, c1, c0) of x2 with
      (n5, c2in, n2) = (p, q, r)

Sharding: data-parallel over batch, 4 samples per core on 8 cores,
processed as 2 groups of 2 samples.

Layout (pair-contiguous): SBUF partition p = b*64 + n//64 =
(b, n5, n4, n3); free j = n % 64 = 16*n2 + 4*n1 + n0, row (i, c).
A 6MB group is contiguous in DRAM and per-partition contiguous in SBUF,
so each group is ONE 2-D in-DMA pair (split in halves so the DVE tree
starts when the first half lands) and ONE 2-D out-DMA.

All reductions are lane-local on DVE (reduced digits live in the free
dim); the PE selector matmuls route/broadcast E (16 matmuls) and
reduce+broadcast G (4 matmuls) across partitions, in bf16 (selector
values 1/64 and 1/4096 are exact in bf16; stage-A sums are cast with
~2^-9 relative error, far inside the 2e-2 budget).  Outputs are
assembled IN-PLACE into the input tile: L0 is the untouched input, L1
is DVE-evacuated from PSUM, and the single 64-float L2 row is
broadcast to all 64 j-rows by the otherwise-idle ACT engine reading
PSUM.  The fully-assembled tile then leaves as ONE contiguous
[128 x 48KB] DMA per group — every out packet is >=16KB-class, vs the
256B strided packets (sub-512B = SDMA read-modify-write penalty) of
the per-region scheme.

All five in-DMAs are pre-issued on gpsimd/SWDGE before any compute
(s12 after group 0's halves so compute starts as early as possible);
group outs ride sync (g0) / scalar (g1) HWDGE so no engine ever
blocks another's issue stream.
"""

import numpy as np

N_CORES = 8
B_FULL = 32
B_CORE = B_FULL // N_CORES  # 4
N = 4096
LVL = 3
C = 64


def _make_selectors():
    """Routing selectors, pair layout: k = 64*b + 16*k5 + 4*k4 + k3.

    S1 block (n2o, c2o), 16 blocks:
        S1[k, m] = 1/64   iff b(k)==b(m), k5==m4, k4==n2o, k3==c2o
    S2 block (c2o), 4 blocks:
        S2[k, m] = 1/4096 iff b(k)==b(m), k5==c2o

    Returned in bfloat16 (both scale factors are powers of two, exact).
    """
    import ml_dtypes

    k = np.arange(128)
    b, k5, k4, k3 = k >> 6, (k >> 4) & 3, (k >> 2) & 3, k & 3
    m = np.arange(128)
    bm, m4 = m >> 6, (m >> 2) & 3
    S1 = np.zeros((128, 16, 128), np.float32)
    S2 = np.zeros((128, 4, 128), np.float32)
    for n2o in range(4):
        for c2o in range(4):
            S1[:, n2o * 4 + c2o, :] = (
                (b[:, None] == bm[None, :])
                & (k5[:, None] == m4[None, :])
                & (k4[:, None] == n2o)
                & (k3[:, None] == c2o)
            ).astype(np.float32) / 64.0
    for c2o in range(4):
        S2[:, c2o, :] = (
            (b[:, None] == bm[None, :]) & (k5[:, None] == c2o)
        ).astype(np.float32) / 4096.0
    bf16 = ml_dtypes.bfloat16
    return (
        np.ascontiguousarray(S1.reshape(128, 2048).astype(bf16)),
        np.ascontiguousarray(S2.reshape(128, 512).astype(bf16)),
    )


def _build_nc():
    import concourse.bass as bass
    import concourse.tile as tile
    from concourse import mybir

    dt = mybir.dt.float32
    bt = mybir.dt.bfloat16
    X = mybir.AxisListType.X
    ADD = mybir.AluOpType.add

    from concourse import bacc
    nc = bacc.Bacc()
    x = nc.declare_dram_parameter("x", [B_CORE, N, LVL, C], dt, isOutput=False)
    s12 = nc.declare_dram_parameter("s12", [128, 2560], bt, isOutput=False)
    out = nc.declare_dram_parameter("out", [B_CORE, N, LVL, C], dt, isOutput=True)

    with tile.TileContext(nc) as tc:
        with (
            tc.tile_pool(name="consts", bufs=1) as cpool,
            tc.tile_pool(name="xin", bufs=2) as xpool,
            tc.tile_pool(name="tmp", bufs=1) as tpool,
            tc.tile_pool(name="psum", bufs=2, space="PSUM") as ppool,
        ):
            # ---- pre-issue every input load on the SWDGE queue ----
            # FIFO order: g0 half1, g0 half2, s12, g1 half1, g1 half2.
            # Group halves split at n2 so the DVE tree starts on half 1.
            xts = []
            for g in range(B_CORE // 2):
                xt = xpool.tile([128, 12288], dt, tag="xt")
                xsrc = x[slice(2 * g, 2 * g + 2)].rearrange(
                    "b (ph j) i c -> (b ph) (j i c)", ph=64
                )
                nc.gpsimd.dma_start(xt[:, 0:6144], xsrc[:, 0:6144])
                nc.gpsimd.dma_start(xt[:, 6144:12288], xsrc[:, 6144:12288])
                xts.append(xt)
                if g == 0:
                    s12sb = cpool.tile([128, 2560], bt, tag="s12")
                    nc.gpsimd.dma_start(s12sb[:], s12[:])
            s1sb = s12sb[:, 0:2048]
            s2sb = s12sb[:, 2048:2560]

            for g in range(B_CORE // 2):
                bs = slice(2 * g, 2 * g + 2)
                xt = xts[g]
                xtv = xt[:].rearrange(
                    "p (j i c) -> p j i c", j=64, i=3, c=64
                )

                # ---- L1 stage A: lane-local sum over (n2, n1, c0) ----
                v = xt[:].rearrange(
                    "p (n2 n1 n0 i c) -> p n2 n1 n0 i c",
                    n2=4, n1=4, n0=4, i=3, c=64,
                )
                u0 = tpool.tile([128, 1024], dt, tag="u0")
                nc.vector.tensor_add(
                    u0[:].rearrange("p (n1 n0 c) -> p n1 n0 c", n1=4, n0=4, c=64),
                    v[:, 0, :, :, 1, :], v[:, 1, :, :, 1, :],
                )
                u1 = tpool.tile([128, 1024], dt, tag="u1")
                nc.vector.tensor_add(
                    u1[:].rearrange("p (n1 n0 c) -> p n1 n0 c", n1=4, n0=4, c=64),
                    v[:, 2, :, :, 1, :], v[:, 3, :, :, 1, :],
                )
                w = tpool.tile([128, 1024], dt, tag="w")
                nc.vector.tensor_add(w[:], u0[:], u1[:])
                h1 = tpool.tile([128, 512], dt, tag="h1")
                nc.vector.tensor_add(h1[:], w[:, 0:512], w[:, 512:1024])
                h2 = tpool.tile([128, 256], dt, tag="h2")
                nc.vector.tensor_add(h2[:], h1[:, 0:256], h1[:, 256:512])
                # reduce c0, write A with free = 16*c2 + 4*c1 + n0
                A = tpool.tile([128, 64], dt, tag="A")
                nc.vector.tensor_reduce(
                    A[:].rearrange("p (c2 c1 n0) -> p n0 c2 c1", c2=4, c1=4, n0=4),
                    h2[:].rearrange(
                        "p (n0 c2 c1 c0) -> p n0 c2 c1 c0", n0=4, c2=4, c1=4, c0=4
                    ),
                    axis=X, op=ADD,
                )
                Ab = tpool.tile([128, 64], bt, tag="Ab")
                nc.vector.tensor_copy(Ab[:], A[:])

                # ---- L2 stage A: lane-local sums ----
                xw = xt[:].rearrange(
                    "p (j i c2 cc) -> p j i c2 cc", j=64, i=3, c2=4, cc=16
                )
                t4 = tpool.tile([128, 256], dt, tag="t4")
                nc.vector.tensor_reduce(
                    t4[:].rearrange("p (j c2) -> p j c2", j=64, c2=4),
                    xw[:, :, 2, :, :],
                    axis=X, op=ADD,
                )
                A2 = tpool.tile([128, 16], dt, tag="A2")
                nc.vector.tensor_reduce(
                    A2[:].rearrange("p (c2 n2) -> p n2 c2", c2=4, n2=4),
                    t4[:].rearrange(
                        "p (n2 nn c2) -> p n2 c2 nn", n2=4, nn=16, c2=4
                    ),
                    axis=X, op=ADD,
                )
                A2b = tpool.tile([128, 16], bt, tag="A2b")
                nc.vector.tensor_copy(A2b[:], A2[:])

                # ---- L1: 16 routing matmuls -> c1p psum (2 banks) ----
                # c1p free = 64*(4*n2o + c2o) + (16*n0o + 4*c1o + c0o)
                c1p = ppool.tile([128, 1024], dt, tag="c1p")
                for n2o in range(4):
                    for c2o in range(4):
                        blk = n2o * 4 + c2o
                        nc.tensor.matmul(
                            c1p[:, blk * 64:(blk + 1) * 64],
                            s1sb[:, blk * 128:(blk + 1) * 128],
                            Ab[:, 0:64],
                            start=True, stop=True,
                        )
                # ---- L2: 4 reduce+broadcast matmuls -> gp psum ----
                # gp free = 16*c2o + (4*c1o + c0o); rhs j = (c2in, n2)
                gp = ppool.tile([128, 64], dt, tag="gp")
                for c2o in range(4):
                    nc.tensor.matmul(
                        gp[:, c2o * 16:(c2o + 1) * 16],
                        s2sb[:, c2o * 128:(c2o + 1) * 128],
                        A2b[:, 0:16],
                        start=True, stop=True,
                    )

                # ---- L1 evac: 16 copies (n2o, n1o), replicate over n1o ----
                c1e = c1p[:].rearrange(
                    "p (n2o c2o n0 cc) -> p n2o c2o n0 cc",
                    n2o=4, c2o=4, n0=4, cc=16,
                )
                xte = xt[:].rearrange(
                    "p (n2 n1 n0 i c2 cc) -> p n2 n1 c2 n0 i cc",
                    n2=4, n1=4, n0=4, i=3, c2=4, cc=16,
                )
                for n2o in range(4):
                    for n1o in range(4):
                        nc.vector.tensor_copy(
                            xte[:, n2o, n1o, :, :, 1, :],
                            c1e[:, n2o, :, :, :],
                        )

                # ---- L2 evac: ACT broadcasts the 64-float G row to all
                # 64 j-rows straight out of PSUM (DVE is busy with the
                # L1 evacuation; ACT is otherwise idle) ----
                nc.scalar.copy(
                    xtv[:, :, 2, :],
                    gp[:].rearrange("p (o c) -> p o c", o=1, c=64)
                    .broadcast_to((128, 64, 64)),
                )

                # ---- out: ONE contiguous DMA per group ----
                # 48KB per partition row -> max-size packets.  g0 rides
                # sync HWDGE, g1 scalar HWDGE (separate rings, and the
                # issuing engine's own waits never block gpsimd).
                outv = out[bs].rearrange("b (ph j) i c -> (b ph) (j i c)", ph=64)
                hw = nc.sync if g == 0 else nc.scalar
                hw.dma_start(outv[:, :], xt[:, :])
    nc.compile()
    return nc


_NC_CACHE = {}


def _get_nc():
    if "nc" not in _NC_CACHE:
        _NC_CACHE["nc"] = _build_nc()
    return _NC_CACHE["nc"]


def kernel(**inputs: np.ndarray) -> np.ndarray:
    from concourse.bass_utils import run_bass_kernel_spmd

    x = np.ascontiguousarray(inputs["x"], dtype=np.float32)
    assert x.shape == (B_FULL, N, LVL, C), x.shape
    S1, S2 = _make_selectors()
    S12 = np.ascontiguousarray(np.concatenate([S1, S2], axis=1))
    nc = _get_nc()
    in_maps = [
        {"x": np.ascontiguousarray(x[k * B_CORE:(k + 1) * B_CORE]),
         "s12": S12}
        for k in range(N_CORES)
    ]
    res = run_bass_kernel_spmd(nc, in_maps, list(range(N_CORES)))
    outs = [res.results[k]["out"] for k in range(N_CORES)]
    return np.ascontiguousarray(np.concatenate(outs, axis=0))
